# revision 53
# baseline (speedup 1.0000x reference)
"""Trainium2 Bass kernel for nn_NoFoDifformer_FourierKAN (8-core SPMD).

Sharding: u and nodes row-wise across 8 cores (1250 rows each). The [d,d]
K^T V Gram matrices and the chunked u^T h partial sums are all-reduced;
small weights are replicated; per-core outputs are produced TRANSPOSED
([d, n_loc]) and transposed+concatenated on the host.

Key structure (v2):
- u is read from HBM exactly once per core via a single SWDGE cast-DMA per
  chunk (fp32 -> bf16 into SBUF); pass-1 matmuls consume the bf16 tiles
  directly, then the same tiles are written (bf16->bf16, HWDGE) to a DRAM
  staging buffer that pass-2 reads back with tall xbar-transposed loads.
- The epilogue (attention apply, residuals, LayerNorms, FFN) runs entirely
  in transposed activation layout [d, i] with weight-stationary matmuls:
  no per-row-tile PE transposes; LN stats via ones-matmul partition sums.
- LayerNorm affine params are folded into downstream projection weights.
"""

import math
from contextlib import ExitStack

import numpy as np

N_FULL = 10000
NF_FULL = 512
D = 128
CORES_FULL = 8
CHUNK_FULL = 1024
LAMBDA_INIT = 0.2


def _ceil_div(a, b):
    return (a + b - 1) // b


def _splits(total, step):
    return [(o, min(step, total - o)) for o in range(0, total, step)]


def build_kernel(N=N_FULL, NF=NF_FULL, CORES=CORES_FULL, CHUNK=CHUNK_FULL,
                 debug=False):
    import concourse.bacc as bacc
    import concourse.tile as tile
    from concourse import mybir
    from concourse.masks import make_identity

    dt = mybir.dt
    f32 = dt.float32
    bf16 = dt.bfloat16
    AF = mybir.ActivationFunctionType
    ALU = mybir.AluOpType
    AX = mybir.AxisListType

    NLOC = N // CORES                  # 1250
    NT = 10                            # row tiles per core
    P = NLOC // NT                     # 125 rows per tile
    assert P * NT == NLOC
    KX = NF // 128                     # x feature k-tiles
    assert KX * 128 == NF
    CHUNKS = _splits(N, CHUNK)         # j chunks
    NCH = len(CHUNKS)
    NSUB = _ceil_div(N, 128)           # global 128-wide j subtiles
    N_PAD = NSUB * 128
    NSUB_C = _ceil_div(CHUNK, 128)     # max j subtiles per chunk
    NLOC_PAD = _ceil_div(NLOC, 16) * 16  # xbar tall-read row pad (1264)
    TG_FULL = N // 128
    TG_REM = N - TG_FULL * 128
    IBLK = _splits(NLOC, 512)          # pass-2 output i blocks
    BLK = _splits(NLOC, 512)           # [128, NLOC] op blocks
    DEPTH = 2                          # pass2 runs DEPTH chunks behind pass1
    assert TG_FULL <= 128
    rg = [list(range(CORES))]
    shared_space = "Shared" if CORES > 4 else "Local"

    nc = bacc.Bacc("TRN2", target_bir_lowering=False, debug=False,
                   num_devices=CORES)

    # ---------------- DRAM I/O ----------------
    def din(name, shape):
        return nc.dram_tensor(name, list(shape), f32, kind="ExternalInput")

    t_x = din("x", (NLOC, NF))
    t_u = din("u", (NLOC, N))
    t_e = din("e", (N,))
    t_few1 = din("fe_w1", (NF, D)); t_feb1 = din("fe_b1", (D,))
    t_few2 = din("fe_w2", (D, D)); t_feb2 = din("fe_b2", (D,))
    t_kana = din("kan_a", (10,)); t_kanb = din("kan_b", (10,))
    t_kanbias = din("kan_bias", (1,)); t_alpha = din("alpha_w", (1, 1))
    t_mg = din("mha_ln_g", (D,)); t_mb = din("mha_ln_b", (D,))
    t_fg = din("ffn_ln_g", (D,)); t_fb = din("ffn_ln_b", (D,))
    t_q1w = din("q1_w", (D, D)); t_q1b = din("q1_b", (D,))
    t_k1w = din("k1_w", (D, D)); t_k1b = din("k1_b", (D,))
    t_q2w = din("q2_w", (D, D)); t_q2b = din("q2_b", (D,))
    t_k2w = din("k2_w", (D, D)); t_k2b = din("k2_b", (D,))
    t_vw = din("v_w", (D, D)); t_vb = din("v_b", (D,))
    t_ag = din("attn_ln_g", (D,)); t_ab = din("attn_ln_b", (D,))
    t_ow = din("out_w", (D, D)); t_ob = din("out_b", (D,))
    t_lq1 = din("lq1", (D,)); t_lk1 = din("lk1", (D,))
    t_lq2 = din("lq2", (D,)); t_lk2 = din("lk2", (D,))
    t_f1w = din("ffn1_w", (D, D)); t_f1b = din("ffn1_b", (D,))
    t_f2w = din("ffn2_w", (D, D)); t_f2b = din("ffn2_b", (D,))
    t_out = nc.dram_tensor("out", [D, NLOC], f32, kind="ExternalOutput")
    if debug:
        bf16_ = __import__("concourse.mybir", fromlist=["dt"]).dt.bfloat16
        t_dhT = nc.dram_tensor("d_hT", [D, NLOC], f32, kind="ExternalOutput")
        t_dhnT = nc.dram_tensor("d_hnT", [D, NLOC], bf16_,
                                kind="ExternalOutput")
        t_dp1 = nc.dram_tensor("d_p1", [D, CHUNK], bf16_,
                               kind="ExternalOutput")
        t_dz = nc.dram_tensor("d_z", [D, _ceil_div(CHUNK, 128) * D], bf16_,
                              kind="ExternalOutput")
        t_duT = nc.dram_tensor("d_uT", [D, _ceil_div(N // CORES, 16) * 16],
                               bf16_, kind="ExternalOutput")
        t_dhaT = nc.dram_tensor("d_haT", [D, NLOC], f32,
                                kind="ExternalOutput")
        t_dhenc = nc.dram_tensor("d_henc", [D, NLOC], f32,
                                 kind="ExternalOutput")
        t_dfT = nc.dram_tensor("d_fT", [D, NLOC], bf16_,
                               kind="ExternalOutput")
        NCH_ = len(_splits(N, CHUNK))
        NLP_ = _ceil_div(N // CORES, 16) * 16
        t_duTall = nc.dram_tensor("d_uTall", [D, NCH_ * NLP_], bf16_,
                                  kind="ExternalOutput")
        t_dzall = nc.dram_tensor("d_zall", [D, NCH_ * D], bf16_,
                                 kind="ExternalOutput")

    with tile.TileContext(nc) as tc, ExitStack() as ctx:
        wpool = ctx.enter_context(tc.tile_pool(name="wpool", bufs=1))
        rowtmp = ctx.enter_context(tc.tile_pool(name="rowtmp", bufs=3))
        ubp = ctx.enter_context(tc.tile_pool(name="ubp", bufs=2))
        uTp = ctx.enter_context(tc.tile_pool(name="uTp", bufs=2))
        z16p = ctx.enter_context(tc.tile_pool(name="z16p", bufs=2))
        p1sbp = ctx.enter_context(
            tc.tile_pool(name="p1sbp", bufs=1 if debug else 2))
        dram = ctx.enter_context(tc.tile_pool(name="dram", bufs=1, space="DRAM"))
        ps_p1 = ctx.enter_context(tc.tile_pool(name="ps_p1", bufs=2, space="PSUM"))
        ps_p2 = ctx.enter_context(tc.tile_pool(name="ps_p2", bufs=3, space="PSUM"))
        ps_mm = ctx.enter_context(tc.tile_pool(name="ps_mm", bufs=2, space="PSUM"))
        ps_g = ctx.enter_context(tc.tile_pool(name="ps_g", bufs=1, space="PSUM"))

        def p1_tile(w):
            return ps_p1.tile([128, 512], f32, tag="p1",
                              name=f"p1_{nc.next_id()}")[:, :w]

        def p2_tile(w):
            return ps_p2.tile([128, 512], f32, tag="p2",
                              name=f"p2_{nc.next_id()}")[:, :w]

        def mm_tile(p, w):
            return ps_mm.tile([128, 512], f32, tag="mmp",
                              name=f"mm_{nc.next_id()}")[:p, :w]

        def wtile(shape, dtype, name):
            return wpool.tile(shape, dtype, tag=name, name=name)

        def rtile(shape, dtype, tag):
            return rowtmp.tile(shape, dtype, tag=tag,
                               name=f"{tag}_{nc.next_id()}")

        def T(out_psum, in_sbuf, identity):
            nc.tensor.matmul(out_psum, in_sbuf, identity, is_transpose=True)

        # ================= constants & weights =================
        ident = wtile([128, 128], f32, "ident")
        make_identity(nc, ident[:])
        identb = wtile([128, 128], bf16, "identb")
        make_identity(nc, identb[:])

        ones_row = wtile([1, 128], f32, "ones_row")
        nc.vector.memset(ones_row[:], 1.0)
        ones_row_b = wtile([1, 128], bf16, "ones_row_b")
        nc.vector.memset(ones_row_b[:], 1.0)
        oinv_col_b = wtile([128, 1], bf16, "oinv_col_b")
        nc.vector.memset(oinv_col_b[:], 1.0 / 128.0)
        eps_col = wtile([128, 1], f32, "eps_col")
        nc.vector.memset(eps_col[:], 1e-5)
        c08_col = wtile([128, 1], f32, "c08_col")
        nc.vector.memset(c08_col[:], 1.0 - LAMBDA_INIT)
        one_col = wtile([128, 1], f32, "one_col")
        nc.vector.memset(one_col[:], 1.0)
        laminit_c = wtile([1, 1], f32, "laminit_c")
        nc.vector.memset(laminit_c[:], LAMBDA_INIT)

        def ldw(name, dram_t, shape, rearr=None, **kw):
            t = wtile(shape, f32, name)
            src = dram_t[:] if rearr is None else dram_t[:].rearrange(rearr, **kw)
            nc.scalar.dma_start(out=t[:], in_=src)
            return t

        few1b = wtile([128, KX, D], bf16, "few1b")
        for kt in range(KX):
            nc.gpsimd.dma_start(out=few1b[:, kt, :],
                                in_=t_few1[kt * 128:(kt + 1) * 128, :])
        few2b = wtile([128, D], bf16, "few2b")
        nc.gpsimd.dma_start(out=few2b[:], in_=t_few2[:])
        f2wb = wtile([128, D], bf16, "f2wb")
        nc.gpsimd.dma_start(out=f2wb[:], in_=t_f2w[:])
        q1w = ldw("q1w", t_q1w, [128, D])
        k1w = ldw("k1w", t_k1w, [128, D])
        q2w = ldw("q2w", t_q2w, [128, D])
        k2w = ldw("k2w", t_k2w, [128, D])
        vw = ldw("vw", t_vw, [128, D])
        ow = ldw("ow", t_ow, [128, D])
        f1w = ldw("f1w", t_f1w, [128, D])

        def ldcol(name, dram_t):
            t = wtile([128, 1], f32, name)
            nc.scalar.dma_start(out=t[:],
                                in_=dram_t[:].rearrange("(p x) -> p x", x=1))
            return t

        feb1_c = ldcol("feb1_c", t_feb1)
        feb2_c = ldcol("feb2_c", t_feb2)
        mg_c = ldcol("mg_c", t_mg); mb_c = ldcol("mb_c", t_mb)
        fg_c = ldcol("fg_c", t_fg); fb_c = ldcol("fb_c", t_fb)
        ag_c = ldcol("ag_c", t_ag); ab_c = ldcol("ab_c", t_ab)
        q1b_c = ldcol("q1b_c", t_q1b); q2b_c = ldcol("q2b_c", t_q2b)
        ob_c = ldcol("ob_c", t_ob)
        f1b_c = ldcol("f1b_c", t_f1b)
        f2b_c = ldcol("f2b_c", t_f2b)

        def ldrow(name, dram_t, w=128):
            t = wtile([1, w], f32, name)
            nc.scalar.dma_start(out=t[:],
                                in_=dram_t[:].rearrange("(x p) -> x p", x=1))
            return t

        k1b_r = ldrow("k1b_r", t_k1b); k2b_r = ldrow("k2b_r", t_k2b)
        vb_r = ldrow("vb_r", t_vb)
        lq1_r = ldrow("lq1_r", t_lq1); lk1_r = ldrow("lk1_r", t_lk1)
        lq2_r = ldrow("lq2_r", t_lq2); lk2_r = ldrow("lk2_r", t_lk2)
        kana_r = ldrow("kana_r", t_kana, 10)
        kanb_r = ldrow("kanb_r", t_kanb, 10)
        kbias_r = ldrow("kbias_r", t_kanbias, 1)
        alpha_r = wtile([1, 1], f32, "alpha_r")
        nc.scalar.dma_start(out=alpha_r[:], in_=t_alpha[:])

        def ldbcast(name, dram_t):
            t = wtile([128, D], f32, name)
            nc.scalar.dma_start(out=t[:], in_=dram_t[:].partition_broadcast(128))
            return t

        feb2_B = ldbcast("feb2_B", t_feb2)

        # ---------- scalars: lambda ----------
        srow = wtile([1, 8], f32, "srow")
        nc.vector.memset(srow[:], 0.0)
        tmpr = wtile([1, 128], f32, "tmpr")
        lam1 = wtile([1, 1], f32, "lam1")
        lam2 = wtile([1, 1], f32, "lam2")
        nc.vector.tensor_mul(tmpr[:], lq1_r[:], lk1_r[:])
        nc.vector.tensor_reduce(lam1[:], tmpr[:], axis=AX.X, op=ALU.add)
        nc.scalar.activation(lam1[:], lam1[:], AF.Exp)
        nc.vector.tensor_mul(tmpr[:], lq2_r[:], lk2_r[:])
        nc.vector.tensor_reduce(lam2[:], tmpr[:], axis=AX.X, op=ALU.add)
        nc.scalar.activation(lam2[:], lam2[:], AF.Exp)
        nc.vector.tensor_sub(srow[:, 0:1], lam1[:], lam2[:])
        nc.vector.tensor_add(srow[:, 0:1], srow[:, 0:1], laminit_c[:])  # lam_full
        nc.scalar.mul(srow[:, 1:2], srow[:, 0:1], -1.0)            # -lam_full
        nc.vector.tensor_copy(srow[:, 2:3], alpha_r[:])
        nc.vector.tensor_copy(srow[:, 3:4], kbias_r[:])

        ps_b = mm_tile(128, 28)
        nc.tensor.matmul(ps_b[:, 0:8], ones_row[:], srow[:],
                         start=True, stop=False)
        nc.tensor.matmul(ps_b[:, 8:18], ones_row[:], kana_r[:],
                         start=False, stop=False)
        nc.tensor.matmul(ps_b[:, 18:28], ones_row[:], kanb_r[:],
                         start=False, stop=True)
        sB = wtile([128, 28], f32, "sB")
        nc.vector.tensor_copy(sB[:], ps_b)
        neglam_c = sB[:, 1:2]
        alpha_c = sB[:, 2:3]
        kbias_c = sB[:, 3:4]

        # ---------- new_e from e (FourierKAN), layout [128, NSUB] ----------
        eT = wtile([128, NSUB], f32, "eT")
        nc.vector.memset(eT[:], 0.0)
        eload = wtile([max(TG_FULL, 1), 128], f32, "eload")
        nc.scalar.dma_start(
            out=eload[:TG_FULL],
            in_=t_e[: TG_FULL * 128].rearrange("(t p) -> t p", p=128))
        pse = mm_tile(128, TG_FULL)
        T(pse, eload[:TG_FULL], ident[:TG_FULL, :TG_FULL])
        nc.vector.tensor_copy(eT[:, :TG_FULL], pse)
        if TG_REM > 0:
            erem = wtile([1, TG_REM], f32, "erem")
            nc.scalar.dma_start(
                out=erem[:],
                in_=t_e[TG_FULL * 128:].rearrange("(x p) -> x p", x=1))
            psr = mm_tile(TG_REM, 1)
            T(psr, erem[:], ident[:1, :1])
            nc.vector.tensor_copy(eT[:TG_REM, TG_FULL:NSUB], psr)

        # Chebyshev recurrence for cos/sin(k*e/pi)
        s1 = wtile([128, NSUB], f32, "s1")
        nc.scalar.activation(s1[:], eT[:], AF.Sin, scale=1.0 / math.pi)
        c1 = wtile([128, NSUB], f32, "c1")
        nc.vector.tensor_mul(c1[:], s1[:], s1[:])
        nc.scalar.activation(c1[:], c1[:], AF.Sqrt, scale=-1.0, bias=1.0)
        twoc = wtile([128, NSUB], f32, "twoc")
        nc.vector.tensor_add(twoc[:], c1[:], c1[:])

        phi = wtile([128, NSUB], f32, "phi")
        ktmp = wtile([128, NSUB], f32, "ktmp")
        nc.vector.tensor_scalar(phi[:], c1[:], scalar1=sB[:, 8:9], scalar2=None, op0=ALU.mult)
        nc.vector.tensor_scalar(ktmp[:], s1[:], scalar1=sB[:, 18:19],
                                scalar2=None, op0=ALU.mult)
        nc.vector.tensor_add(phi[:], phi[:], ktmp[:])
        cp, sp = c1, s1
        cpp, spp = None, None
        for k in range(2, 11):
            ck = rtile([128, NSUB], f32, "ckt")
            sk = rtile([128, NSUB], f32, "skt")
            nc.vector.tensor_mul(ck[:], twoc[:], cp[:])
            nc.vector.tensor_mul(sk[:], twoc[:], sp[:])
            if k == 2:
                nc.vector.tensor_scalar(ck[:], ck[:], scalar1=one_col[:],
                                        scalar2=None, op0=ALU.subtract)
            else:
                nc.vector.tensor_sub(ck[:], ck[:], cpp[:])
                nc.vector.tensor_sub(sk[:], sk[:], spp[:])
            nc.vector.tensor_scalar(ktmp[:], ck[:],
                                    scalar1=sB[:, 7 + k:8 + k], scalar2=None, op0=ALU.mult)
            nc.vector.tensor_add(phi[:], phi[:], ktmp[:])
            nc.vector.tensor_scalar(ktmp[:], sk[:],
                                    scalar1=sB[:, 17 + k:18 + k], scalar2=None, op0=ALU.mult)
            nc.vector.tensor_add(phi[:], phi[:], ktmp[:])
            cpp, spp = cp, sp
            cp, sp = ck, sk
        ne = wtile([128, NSUB], f32, "ne")
        nc.vector.tensor_scalar(ne[:], phi[:], scalar1=kbias_c, op0=ALU.add,
                                scalar2=alpha_c, op1=ALU.mult)

        # ---------- folded weights (LN affine into projections) ----------
        def fold_w(name, w_sb, g_col):
            t = wtile([128, D], bf16, name)
            nc.vector.tensor_scalar(t[:], w_sb[:], scalar1=g_col[:], scalar2=None, op0=ALU.mult)
            return t

        Wk1b = fold_w("Wk1b", k1w, mg_c); Wk2b = fold_w("Wk2b", k2w, mg_c)
        Wvb = fold_w("Wvb", vw, mg_c)
        Wq1 = wtile([128, D], f32, "Wq1")
        nc.vector.tensor_scalar(Wq1[:], q1w[:], scalar1=mg_c[:], scalar2=None, op0=ALU.mult)
        Wq2 = wtile([128, D], f32, "Wq2")
        nc.vector.tensor_scalar(Wq2[:], q2w[:], scalar1=mg_c[:], scalar2=None, op0=ALU.mult)
        W1pb = fold_w("W1pb", f1w, fg_c)
        Wob = wtile([128, D], bf16, "Wob")
        nc.vector.tensor_scalar(Wob[:], ow[:], scalar1=ag_c[:], op0=ALU.mult,
                                scalar2=c08_col[:], op1=ALU.mult)


        def fold_b(name, w_sb, beta_col, b_row):
            # row [1, D] bias: beta^T @ W + b
            psb = mm_tile(1, D)
            nc.tensor.matmul(psb, beta_col[:], w_sb[:])
            t = wtile([1, D], f32, name)
            nc.vector.tensor_add(t[:], psb, b_row[:])
            return t

        bk1_r = fold_b("bk1_r", k1w, mb_c, k1b_r)
        bk2_r = fold_b("bk2_r", k2w, mb_c, k2b_r)
        bv_r = fold_b("bv_r", vw, mb_c, vb_r)
        psq = mm_tile(128, 1)
        nc.tensor.matmul(psq, q1w[:], mb_c[:])
        bq1_c = wtile([128, 1], f32, "bq1_c")
        nc.vector.tensor_add(bq1_c[:], psq, q1b_c[:])
        psq2 = mm_tile(128, 1)
        nc.tensor.matmul(psq2, q2w[:], mb_c[:])
        bq2_c = wtile([128, 1], f32, "bq2_c")
        nc.vector.tensor_add(bq2_c[:], psq2, q2b_c[:])
        # column biases for transposed epilogue
        pso = mm_tile(128, 1)
        nc.tensor.matmul(pso, ow[:], ab_c[:])
        bo_c = wtile([128, 1], f32, "bo_c")
        nc.vector.tensor_scalar(bo_c[:], pso, scalar1=c08_col[:], scalar2=None, op0=ALU.mult)
        nc.vector.tensor_add(bo_c[:], bo_c[:], ob_c[:])
        psp1 = mm_tile(128, 1)
        nc.tensor.matmul(psp1, f1w[:], fb_c[:])
        b1p_c = wtile([128, 1], f32, "b1p_c")
        nc.vector.tensor_add(b1p_c[:], psp1, f1b_c[:])

        def bcast_row(name, row_sb):
            psb = mm_tile(128, D)
            nc.tensor.matmul(psb, ones_row[:], row_sb[:])
            t = wtile([128, D], f32, name)
            nc.vector.tensor_copy(t[:], psb)
            return t

        bk1_B = bcast_row("bk1_B", bk1_r)
        bk2_B = bcast_row("bk2_B", bk2_r)
        bv_B = bcast_row("bv_B", bv_r)

        # ---------- DRAM staging ----------
        # per-chunk u16 tensors: a shared tensor would create false WAR
        # edges between chunk-c writes and chunk-(c-1) transposed reads
        u16c = [dram.tile([NLOC_PAD, _ceil_div(cw, 128) * 128], bf16,
                          tag=f"u16_{c}", name=f"u16_{c}")
                for c, (co, cw) in enumerate(CHUNKS)]
        p1_in, p1_out = [], []
        for c, (co, cw) in enumerate(CHUNKS):
            p1_in.append(dram.tile([128, cw], bf16, tag=f"p1in{c}",
                                   name=f"p1in{c}"))
            p1_out.append(dram.tile([128, cw], bf16, tag=f"p1out{c}",
                                    name=f"p1out{c}", addr_space=shared_space))
        gr_in = dram.tile([128, 2 * D], f32, tag="gr_in", name="gr_in")
        gr_out = dram.tile([128, 2 * D], f32, tag="gr_out", name="gr_out",
                           addr_space=shared_space)

        # ---------- u chunk cast loads (SWDGE fp32 -> bf16) ----------
        ub_tiles = {}

        def emit_cast_load(c):
            co, cw = CHUNKS[c]
            ub = ubp.tile([P, NT, CHUNK], bf16, tag="ub", name=f"ub{c}")
            for r in range(NT):
                nc.gpsimd.dma_start(
                    out=ub[:, r, :cw],
                    in_=t_u[r * P:(r + 1) * P, co:co + cw])
            ub_tiles[c] = ub

        emit_cast_load(0)
        emit_cast_load(1)

        # ================= phase A: feature encoder =================
        h1Tb = wtile([128, NLOC], bf16, "h1Tb")
        for go, gw in _splits(NLOC, 4 * P):
            xTg = rowtmp.tile([128, KX, 4 * P], bf16, tag="xTg", bufs=2,
                              name=f"xTg_{nc.next_id()}")
            for ro in range(0, gw, P):
                xt = rowtmp.tile([P, NF], f32, tag="xt", bufs=3,
                                 name=f"xt_{nc.next_id()}")
                nc.sync.dma_start(out=xt[:], in_=t_x[go + ro:go + ro + P, :])
                for kt in range(KX):
                    pst = mm_tile(128, P)
                    T(pst, xt[:, kt * 128:(kt + 1) * 128], ident[:P, :P])
                    nc.vector.tensor_copy(xTg[:, kt, ro:ro + P], pst)
            psh1 = p2_tile(gw)
            for kt in range(KX):
                nc.tensor.matmul(psh1, few1b[:, kt, :], xTg[:, kt, :gw],
                                 start=(kt == 0), stop=(kt == KX - 1))
            nc.scalar.activation(h1Tb[:, go:go + gw], psh1, AF.Relu,
                                 bias=feb1_c[:])

        hT = wtile([128, NLOC], f32, "hT")
        for bo, bw in BLK:
            psh = p2_tile(bw)
            nc.tensor.matmul(psh, few2b[:], h1Tb[:, bo:bo + bw])
            nc.vector.tensor_scalar(hT[:, bo:bo + bw], psh, scalar1=feb2_c[:],
                                    scalar2=None, op0=ALU.add)

        def dbg_dump(dst_dram, src_ap, width, off=0):
            nc.sync.dma_start(out=dst_dram[:, off:off + width],
                              in_=src_ap[:, :width])

        if debug:
            dbg_dump(t_dhT, hT, NLOC)

        # ---- per-row-tile: h16 rows, LN, hnT, k/v projections, gram ----
        def layer_norm(src_ap, rw, out_ap):
            stats = rtile([128, 6], f32, "stats")
            nc.vector.bn_stats(stats[:rw], src_ap)
            mv = rtile([128, 2], f32, "mv")
            nc.vector.bn_aggr(mv[:rw], stats[:rw])
            rs = rtile([128, 1], f32, "rs")
            nc.scalar.activation(rs[:rw], mv[:rw, 1:2], AF.Sqrt,
                                 bias=eps_col[:rw])
            nc.vector.reciprocal(rs[:rw], rs[:rw])
            nc.vector.tensor_scalar(out_ap, src_ap, scalar1=mv[:rw, 0:1],
                                    op0=ALU.subtract, scalar2=rs[:rw],
                                    op1=ALU.mult)

        h16 = wtile([P, NT, D], bf16, "h16")
        hnTb = wtile([128, NLOC], bf16, "hnTb")
        # gram computed transposed:  psg = v^T @ [k1 | k2]  (one PSUM bank)
        psg = ps_g.tile([128, 2 * D], f32, tag="g", name="psg")
        for r in range(NT):
            ro = r * P
            psr = mm_tile(P, D)
            nc.tensor.matmul(psr, h1Tb[:, ro:ro + P], few2b[:])
            hrow = rtile([P, D], f32, "hrow")
            nc.vector.tensor_add(hrow[:], psr, feb2_B[:P])
            nc.vector.tensor_add(h16[:, r, :], psr, feb2_B[:P])
            hn = rtile([P, D], f32, "hn")
            layer_norm(hrow[:], P, hn[:])
            psT = mm_tile(128, P)
            T(psT, hn[:], ident[:P, :P])
            nc.vector.tensor_copy(hnTb[:, ro:ro + P], psT)
            k12t = rtile([P, 2, D], bf16, "k12t")
            vt = rtile([P, D], bf16, "vt")
            for dst, W, bB in ((k12t[:, 0, :], Wk1b, bk1_B),
                               (k12t[:, 1, :], Wk2b, bk2_B),
                               (vt[:], Wvb, bv_B)):
                psp = mm_tile(P, D)
                nc.tensor.matmul(psp, hnTb[:, ro:ro + P], W[:])
                nc.vector.tensor_add(dst, psp, bB[:P])
            nc.tensor.matmul(psg[:], vt[:], k12t[:, :, :],
                             start=(r == 0), stop=(r == NT - 1))

        if debug:
            dbg_dump(t_dhnT, hnTb, NLOC)

        gram = wtile([128, 2 * D], f32, "gram")
        nc.vector.tensor_copy(gram[:], psg[:])
        nc.gpsimd.dma_start(out=gr_in[:], in_=gram[:])
        nc.gpsimd.collective_compute("AllReduce", ALU.add, replica_groups=rg,
                                     ins=[gr_in.opt()], outs=[gr_out.opt()])

        # ---------- transposed-layout LayerNorm helper ----------
        def lnT(x_sb, out_bf, pfx):
            xb = wpool.tile([128, NLOC], bf16, tag="ln_xb",
                            name=f"{pfx}_xb")
            nc.vector.tensor_copy(xb[:], x_sb[:])
            x2b = wpool.tile([128, NLOC], bf16, tag="ln_x2b",
                             name=f"{pfx}_x2b")
            nc.vector.tensor_mul(x2b[:], x_sb[:], x_sb[:])
            mrow = wpool.tile([1, NLOC], f32, tag="ln_m", name=f"{pfx}_m")
            qrow = wpool.tile([1, NLOC], f32, tag="ln_q", name=f"{pfx}_q")
            for bo, bw in BLK:
                psm = mm_tile(1, bw)
                nc.tensor.matmul(psm, oinv_col_b[:], xb[:, bo:bo + bw])
                nc.vector.tensor_copy(mrow[:, bo:bo + bw], psm)
                psq_ = mm_tile(1, bw)
                nc.tensor.matmul(psq_, oinv_col_b[:], x2b[:, bo:bo + bw])
                nc.vector.tensor_copy(qrow[:, bo:bo + bw], psq_)
            m2 = wpool.tile([1, NLOC], f32, tag="ln_m2", name=f"{pfx}_m2")
            nc.vector.tensor_mul(m2[:], mrow[:], mrow[:])
            nc.vector.tensor_sub(m2[:], qrow[:], m2[:])          # var
            nc.scalar.activation(m2[:], m2[:], AF.Sqrt, bias=eps_col[:1])
            nc.vector.reciprocal(m2[:], m2[:])                   # rs
            m_b = wpool.tile([1, NLOC], bf16, tag="ln_mb", name=f"{pfx}_mb")
            nc.vector.tensor_copy(m_b[:], mrow[:])
            rs_b = wpool.tile([1, NLOC], bf16, tag="ln_rb", name=f"{pfx}_rb")
            nc.vector.tensor_copy(rs_b[:], m2[:])
            for bo, bw in BLK:
                psM = mm_tile(128, bw)
                nc.tensor.matmul(psM, ones_row_b[:], m_b[:, bo:bo + bw])
                psR = mm_tile(128, bw)
                nc.tensor.matmul(psR, ones_row_b[:], rs_b[:, bo:bo + bw])
                dtmp = rowtmp.tile([128, 512], f32, tag="lnd", bufs=2,
                                   name=f"lnd_{nc.next_id()}")[:, :bw]
                nc.vector.tensor_sub(dtmp, x_sb[:, bo:bo + bw], psM)
                nc.vector.tensor_mul(out_bf[:, bo:bo + bw], dtmp, psR)

        # ================= chunk pipeline =================
        henc = wtile([128, NLOC], f32, "henc")
        haT = wtile([128, NLOC], f32, "haT")
        sT = wtile([128, NLOC], f32, "sT")
        aTb = wtile([128, NLOC], bf16, "aTb")

        def emit_u16_write(c):
            co, cw = CHUNKS[c]
            for r in range(NT):
                nc.scalar.dma_start(
                    out=u16c[c][r * P:(r + 1) * P, :cw],
                    in_=ub_tiles[c][:, r, :cw])

        def emit_pass1(c):
            co, cw = CHUNKS[c]
            blocks = _splits(cw, 512)
            ps1 = [p1_tile(bw) for bo, bw in blocks]
            for r in range(NT):
                for b, (bo, bw) in enumerate(blocks):
                    nc.tensor.matmul(ps1[b], h16[:, r, :],
                                     ub_tiles[c][:, r, bo:bo + bw],
                                     start=(r == 0), stop=(r == NT - 1))
            p1sb = p1sbp.tile([128, CHUNK], bf16, tag="p1sb",
                              name=f"p1sb{c}")[:, :cw]
            for b, (bo, bw) in enumerate(blocks):
                nc.vector.tensor_copy(p1sb[:, bo:bo + bw], ps1[b])
            nc.gpsimd.dma_start(out=p1_in[c][:], in_=p1sb)
            nc.gpsimd.collective_compute(
                "AllReduce", ALU.add, replica_groups=rg,
                ins=[p1_in[c].opt()], outs=[p1_out[c].opt()])
            if debug and c == 0:
                dbg_dump(t_dp1, p1sb, cw)
            del ub_tiles[c]

        uT_tiles = {}

        def emit_uT_reads(c):
            co, cw = CHUNKS[c]
            nsub_c = _ceil_div(cw, 128)
            tiles = []
            for t in range(nsub_c):
                uTt = uTp.tile([128, NLOC_PAD], bf16, tag="uT",
                               bufs=2 * NSUB_C, name=f"uT{c}_{t}")
                nc.sync.dma_start(out=uTt[:],
                                  in_=u16c[c][:, t * 128:(t + 1) * 128],
                                  transpose=True)
                tiles.append(uTt)
            uT_tiles[c] = tiles

        def emit_pass2(c):
            co, cw = CHUNKS[c]
            subs = _splits(cw, 128)
            z16 = z16p.tile([128, NSUB_C, D], bf16, tag="z16", name=f"z16_{c}")
            for t, (so, sw) in enumerate(subs):
                zr = rowtmp.tile([128, D], bf16, tag="zr", bufs=3,
                                 name=f"zr_{nc.next_id()}")
                if sw % 128 == 0:
                    nc.sync.dma_start(out=zr[:sw, :],
                                      in_=p1_out[c][:, so:so + sw],
                                      transpose=True)
                else:
                    nc.scalar.dma_start(
                        out=zr[:sw, :],
                        in_=p1_out[c][:, so:so + sw].rearrange("a b -> b a"))
                gidx = (co + so) // 128
                nc.vector.tensor_scalar(z16[:sw, t, :], zr[:sw, :],
                                        scalar1=ne[:sw, gidx:gidx + 1],
                                        scalar2=None, op0=ALU.mult)
            uTc = uT_tiles.pop(c)
            if debug and c == 0:
                dbg_dump(t_duT, uTc[0], NLOC_PAD)
                for t, (so, sw) in enumerate(subs):
                    dbg_dump(t_dz, z16[:, t, :], D, off=t * D)
            if debug:
                dbg_dump(t_duTall, uTc[0], NLOC_PAD, off=c * NLOC_PAD)
                dbg_dump(t_dzall, z16[:, 0, :], D, off=c * D)
            ps2 = [p2_tile(iw) for io, iw in IBLK]
            for t, (so, sw) in enumerate(subs):
                for ib, (io, iw) in enumerate(IBLK):
                    nc.tensor.matmul(ps2[ib], z16[:sw, t, :],
                                     uTc[t][:sw, io:io + iw],
                                     start=(t == 0),
                                     stop=(t == len(subs) - 1))
            for ib, (io, iw) in enumerate(IBLK):
                if c == 0:
                    nc.vector.tensor_copy(henc[:, io:io + iw], ps2[ib])
                else:
                    nc.vector.tensor_add(henc[:, io:io + iw],
                                         henc[:, io:io + iw], ps2[ib])

        def emit_watt():
            # kv holds (v^T k1 | v^T k2) = (k1v^T | k2v^T)
            kv = wtile([128, 2 * D], f32, "kv")
            nc.scalar.dma_start(out=kv[:], in_=gr_out[:])
            psk1 = mm_tile(128, 128)
            T(psk1, kv[:, :D], ident[:])
            k1vs = wtile([128, D], f32, "k1vs")
            nc.vector.tensor_copy(k1vs[:], psk1)
            psk2 = mm_tile(128, 128)
            T(psk2, kv[:, D:], ident[:])
            k2vs = wtile([128, D], f32, "k2vs")
            nc.vector.tensor_copy(k2vs[:], psk2)
            psq1T = mm_tile(128, 128)
            T(psq1T, Wq1[:], ident[:])
            Wq1T = wtile([128, D], f32, "Wq1T")
            nc.vector.tensor_copy(Wq1T[:], psq1T)
            psq2T = mm_tile(128, 128)
            T(psq2T, Wq2[:], ident[:])
            Wq2T = wtile([128, D], f32, "Wq2T")
            nc.vector.tensor_copy(Wq2T[:], psq2T)

            ps_w1e = mm_tile(D, D)
            nc.tensor.matmul(ps_w1e, Wq1T[:], k1vs[:])
            ps_w2e = mm_tile(D, D)
            nc.tensor.matmul(ps_w2e, Wq2T[:], k2vs[:])
            Watt = wtile([128, D], f32, "Watt")
            nc.vector.tensor_scalar(Watt[:], ps_w2e, scalar1=neglam_c,
                                    scalar2=None, op0=ALU.mult)
            nc.vector.tensor_add(Watt[:], Watt[:], ps_w1e)
            Wattb = wtile([128, D], bf16, "Wattb")
            nc.vector.tensor_copy(Wattb[:], Watt[:])

            ps_b1 = mm_tile(128, 1)
            nc.tensor.matmul(ps_b1, k1vs[:], bq1_c[:])
            ps_b2 = mm_tile(128, 1)
            nc.tensor.matmul(ps_b2, k2vs[:], bq2_c[:])
            batt_c = wtile([128, 1], f32, "batt_c")
            nc.vector.tensor_scalar(batt_c[:], ps_b2, scalar1=neglam_c,
                                    scalar2=None, op0=ALU.mult)
            nc.vector.tensor_add(batt_c[:], batt_c[:], ps_b1)
            return Wattb, batt_c

        for c in range(NCH):
            if c + 2 < NCH:
                emit_cast_load(c + 2)
            emit_u16_write(c)
            emit_pass1(c)
            if c == 1:
                Wattb, batt_c = emit_watt()
            if c == 2:
                # sT = Watt^T @ hnT + batt  (transposed layout)
                for bo, bw in BLK:
                    pss = mm_tile(128, bw)
                    nc.tensor.matmul(pss, Wattb[:], hnTb[:, bo:bo + bw])
                    nc.vector.tensor_scalar(sT[:, bo:bo + bw], pss,
                                            scalar1=batt_c[:], scalar2=None,
                                            op0=ALU.add)
                lnT(sT, aTb, "s")
            if c == 3:
                # haT = hT + Wo'^T @ aT + bo
                for bo, bw in BLK:
                    psa = mm_tile(128, bw)
                    nc.tensor.matmul(psa, Wob[:], aTb[:, bo:bo + bw])
                    atmp = rowtmp.tile([128, 512], f32, tag="atmp", bufs=2,
                                       name=f"atmp_{nc.next_id()}")[:, :bw]
                    nc.vector.tensor_scalar(atmp, psa, scalar1=bo_c[:],
                                            scalar2=None, op0=ALU.add)
                    nc.vector.tensor_add(haT[:, bo:bo + bw],
                                         hT[:, bo:bo + bw], atmp)
                if debug:
                    dbg_dump(t_dhaT, haT, NLOC)
            if c >= DEPTH:
                emit_pass2(c - DEPTH)
            if c >= 1:
                emit_uT_reads(c - 1)
        emit_uT_reads(NCH - 1)

        for c in range(NCH - DEPTH, NCH):
            emit_pass2(c)

        # ================= transposed epilogue =================
        if debug:
            dbg_dump(t_dhenc, henc, NLOC)
        nc.vector.tensor_add(haT[:], haT[:], henc[:])            # mhT
        fTb = wtile([128, NLOC], bf16, "fTb")
        lnT(haT, fTb, "f")
        if debug:
            dbg_dump(t_dfT, fTb, NLOC)
        gb = wtile([128, NLOC], bf16, "gb")
        for bo, bw in BLK:
            psg_ = mm_tile(128, bw)
            nc.tensor.matmul(psg_, W1pb[:], fTb[:, bo:bo + bw])
            nc.scalar.activation(gb[:, bo:bo + bw], psg_, AF.Gelu,
                                 bias=b1p_c[:])
        outT = sT  # sT is dead after lnT(sT); reuse its buffer
        for bo, bw in BLK:
            pso_ = mm_tile(128, bw)
            nc.tensor.matmul(pso_, f2wb[:], gb[:, bo:bo + bw])
            otmp = rowtmp.tile([128, 512], f32, tag="otmp", bufs=2,
                               name=f"otmp_{nc.next_id()}")[:, :bw]
            nc.vector.tensor_scalar(otmp, pso_, scalar1=f2b_c[:],
                                    scalar2=None, op0=ALU.add)
            nc.vector.tensor_add(outT[:, bo:bo + bw],
                                 haT[:, bo:bo + bw], otmp)
        nc.sync.dma_start(out=t_out[:], in_=outT[:])

    nc.compile()
    return nc


# ==================== host-side entry point ====================

_CACHED = {}


def _get_nc(N=N_FULL, NF=NF_FULL, CORES=CORES_FULL, CHUNK=CHUNK_FULL):
    key = (N, NF, CORES, CHUNK)
    if key not in _CACHED:
        _CACHED[key] = build_kernel(N, NF, CORES, CHUNK)
    return _CACHED[key]


def make_in_maps(inputs, N, CORES):
    NLOC = N // CORES
    full = {k: np.ascontiguousarray(np.asarray(v, dtype=np.float32))
            for k, v in inputs.items()}
    in_maps = []
    for c in range(CORES):
        m = {}
        for k, v in full.items():
            if k in ("x", "u"):
                m[k] = np.ascontiguousarray(v[c * NLOC:(c + 1) * NLOC])
            else:
                m[k] = v
        in_maps.append(m)
    return in_maps


def assemble_out(res, CORES=CORES_FULL):
    # per-core outputs are [D, NLOC] (transposed); transpose + concat rows
    return np.concatenate(
        [np.asarray(res.results[c]["out"]).T for c in range(CORES)],
        axis=0).astype(np.float32)


def kernel(**inputs):
    from concourse import bass_utils

    nc = _get_nc()
    in_maps = make_in_maps(inputs, N_FULL, CORES_FULL)
    res = bass_utils.run_bass_kernel_spmd(nc, in_maps,
                                          core_ids=list(range(CORES_FULL)))
    return assemble_out(res)


if __name__ == "__main__":
    build_kernel()
    print("build ok")
